# revision 13
# baseline (speedup 1.0000x reference)
"""GPT forward (L=6, B=2, T=1024, D=768, H=12, V=50257) on 8 TRN2 NeuronCores.

Sharding: queries 32-interleaved (core i of each 4-core batch group owns the
i-th 32-token sub-block of every 128-token block), weights replicated.  This
makes block-causal skipping core-uniform: every core computes exactly the
9 causally-live (q-chunk, key-block) pair units instead of all 20.

Per layer the K/V AllGather is split into 3 head-group chunks that pipeline
with attention (chunk c feeds head-pairs 2c, 2c+1), keeping the PE busy
through the collective.  AV runs V-stationary and column-packed (two heads
per matmul, out = V.T @ A feature-major), so attention output needs no PE
transposes.  Softmax denominators come from ones-column matmuls packed into
spare PSUM partitions; LN rstd is computed as Exp(-0.5*Ln(var+eps)) so the
whole layer uses one ACT table set plus gelu.

All matmul operands are bf16; residual stream and LN stats stay fp32.
"""
import os
import numpy as np
from contextlib import ExitStack

import concourse.bass as bass
import concourse.tile as tile
import concourse.mybir as mybir
from concourse import bacc, bass_utils

F32 = mybir.dt.float32
F32R = mybir.dt.float32r
BF16 = mybir.dt.bfloat16
AF = mybir.ActivationFunctionType
OP = mybir.AluOpType

L, B, T, D, H, DK, V = 6, 2, 1024, 768, 12, 64, 50257
NB, TB, TPC = 8, 128, 256
NJ = D // 128                        # 6
NJ1 = 4 * D // 128                   # 24
VCHUNK = 512
NVC = 13
VCP = NVC * VCHUNK                   # 6656
VC = 6283                            # 8*6283 = 50264 >= V
EPS = 1e-5
NMT = 16
NLAYER = int(os.environ.get("KLAYERS", str(L)))

# slot layout inside an A tile: pair g at 512*g, tight within the pair
WS = [256 - 32 * s for s in range(8)]            # slot widths
OS = [0, 256, 512, 704, 1024, 1152, 1536, 1600]  # slot offsets
GW = [480, 352, 224, 96]                         # pair-group widths
AW = 2048                                        # A tile width (4 x 512)


def _build():
    nc = bacc.Bacc("TRN2", target_bir_lowering=False, debug=False)

    di = {}
    def din(name, shape, dt=F32R):
        di[name] = nc.dram_tensor(name, shape, dt, kind="ExternalInput")
        return di[name]

    din("x0T", [128, NJ * TPC])
    din("cosT", [128, NJ * TPC], BF16)
    din("sinS", [128, NJ * TPC], BF16)
    din("dmask4", [128, 4 * 32], BF16)
    din("onecol", [128, 1])
    din("onesrow", [1, 128])
    din("onesbf", [128, 1], BF16)
    din("sel_e", [128, 128])
    din("sel_o", [128, 128])
    din("embT", [D, VCP], BF16)
    for nm in ("Wq", "Wk", "Wv", "Wo"):
        din(nm, [L, D, D], BF16)
    din("W1", [L, D, 4 * D], BF16)
    din("W2", [L, 4 * D, D], BF16)
    for nm in ("bq_p", "bk_p", "bo_p", "b2_p", "g_p", "be_p", "l2w_p", "l2b_p"):
        din(nm, [L, 128, NJ], F32)
    din("b1_p", [L, 128, NJ1], F32)
    din("bv_bc", [L, 128, D], F32)
    din("lnw_p", [128, NJ], F32)
    din("lnb_p", [128, NJ], F32)

    out_logits = nc.dram_tensor("logits", [NMT * 128, VCP], BF16,
                                kind="ExternalOutput")

    with tile.TileContext(nc) as tc, ExitStack() as octx:
        const = octx.enter_context(tc.tile_pool(name="const", bufs=1))
        xpool = octx.enter_context(tc.tile_pool(name="x", bufs=1))
        small = octx.enter_context(tc.tile_pool(name="small", bufs=4))
        bias = octx.enter_context(tc.tile_pool(name="bias", bufs=2))
        pp = octx.enter_context(tc.tile_pool(name="pp", bufs=8, space="PSUM"))
        dram = octx.enter_context(tc.tile_pool(name="dram", bufs=2, space="DRAM"))

        t_ones = const.tile([128, 1], F32R, tag="ones")
        nc.sync.dma_start(t_ones[:], di["onecol"].ap())
        t_onesr = const.tile([1, 128], F32R, tag="onesr")
        nc.sync.dma_start(t_onesr[:], di["onesrow"].ap())
        t_onesbf = const.tile([128, 1], BF16, tag="onesbf")
        nc.sync.dma_start(t_onesbf[:], di["onesbf"].ap())
        t_sele = const.tile([128, 128], F32R, tag="sele")
        nc.sync.dma_start(t_sele[:], di["sel_e"].ap())
        t_selo = const.tile([128, 128], F32R, tag="selo")
        nc.sync.dma_start(t_selo[:], di["sel_o"].ap())
        t_lnw = const.tile([128, NJ], F32, tag="lnw")
        nc.sync.dma_start(t_lnw[:], di["lnw_p"].ap())
        t_lnb = const.tile([128, NJ], F32, tag="lnb")
        nc.sync.dma_start(t_lnb[:], di["lnb_p"].ap())
        t_eps = const.tile([1, 1], F32, tag="eps")
        nc.gpsimd.memset(t_eps[:], EPS)

        t_x = xpool.tile([128, NJ * TPC], F32R, tag="x")
        nc.sync.dma_start(t_x[:], di["x0T"].ap())
        t_hT = xpool.tile([128, NJ * TPC], BF16, tag="hT")

        pcnt = [0]

        def psum(w=TPC):
            pcnt[0] += 1
            return pp.tile([128, w], F32, tag="pp", name=f"ps{pcnt[0]}")

        def psum1(w=TPC):
            pcnt[0] += 1
            return pp.tile([1, w], F32, tag="pp", name=f"ps{pcnt[0]}")

        def layernorm(wpool, src, dst, gt, bt, pre=None):
            """feature-major LN: dst(bf16) = (src - mean)/std * g + b.
            pre(j) runs before chunk j's stats (fused residual eviction).
            rstd = exp(-0.5*ln(var+eps)) -- stays in the exp/ln ACT set."""
            t_sq = wpool.tile([128, NJ * TPC], F32R, tag="scratch6")
            p_s = psum1()
            p_q = psum1()
            for j in range(NJ):
                if pre is not None:
                    pre(j)
                sl = slice(j * TPC, (j + 1) * TPC)
                nc.gpsimd.tensor_tensor(t_sq[:, sl], src[:, sl], src[:, sl],
                                        OP.mult)
                nc.tensor.matmul(p_s[:], t_ones[:], src[:, sl],
                                 start=(j == 0), stop=(j == NJ - 1))
                nc.tensor.matmul(p_q[:], t_ones[:], t_sq[:, sl],
                                 start=(j == 0), stop=(j == NJ - 1))
            # ones vector holds 1/D, so p_s = mean and p_q = E[x^2] directly
            t_mean = small.tile([1, TPC], F32R, tag="mean")
            nc.vector.tensor_copy(t_mean[:], p_s[:])
            t_msq = small.tile([1, TPC], F32, tag="msq")
            nc.vector.tensor_tensor(t_msq[:], t_mean[:], p_s[:], OP.mult)
            t_var = small.tile([1, TPC], F32, tag="var")
            nc.vector.tensor_tensor(t_var[:], p_q[:], t_msq[:], OP.subtract)
            t_lv = small.tile([1, TPC], F32, tag="lv")
            nc.scalar.activation(t_lv[:], t_var[:], AF.Ln, bias=t_eps[:])
            t_rstd = small.tile([1, TPC], F32R, tag="rstd")
            with nc.allow_low_precision(reason="f32r view of f32 for PE bcast"):
                nc.scalar.activation(t_rstd[:], t_lv[:], AF.Exp, scale=-0.5)
            t_mb = small.tile([128, TPC], F32, tag="mb")
            p_mb = psum()
            nc.tensor.matmul(p_mb[:], t_onesr[:], t_mean[:])
            nc.vector.tensor_copy(t_mb[:], p_mb[:])
            t_rb = small.tile([128, TPC], F32, tag="rb")
            p_rb = psum()
            nc.tensor.matmul(p_rb[:], t_onesr[:], t_rstd[:])
            nc.vector.tensor_copy(t_rb[:], p_rb[:])
            t_c = wpool.tile([128, NJ * TPC], F32, tag="lnc")
            for j in range(NJ):
                sl = slice(j * TPC, (j + 1) * TPC)
                eng = nc.vector if j % 2 == 0 else nc.gpsimd
                eng.tensor_tensor(t_c[:, sl], src[:, sl], t_mb[:], OP.subtract)
                eng.tensor_tensor(dst[:, sl], t_c[:, sl], t_rb[:], OP.mult)
                eng.tensor_scalar(dst[:, sl], dst[:, sl], gt[:, j:j + 1],
                                  bt[:, j:j + 1], OP.mult, OP.add)

        def rope(wpool, t_q, t_cos, t_sin, c0, w, eng=None):
            """in-place RoPE on feature-major bf16 cols [c0, c0+w).
            Partition swap via SBUF->SBUF DMAs on two queues."""
            eng = eng or nc.vector
            t_sw = wpool.tile([128, NJ * TPC], BF16, tag="ropesw")
            nc.sync.dma_start(t_sw[0:32, c0:c0 + w], t_q[32:64, c0:c0 + w])
            nc.gpsimd.dma_start(t_sw[32:64, c0:c0 + w], t_q[0:32, c0:c0 + w])
            nc.gpsimd.dma_start(t_sw[64:96, c0:c0 + w], t_q[96:128, c0:c0 + w])
            nc.sync.dma_start(t_sw[96:128, c0:c0 + w], t_q[64:96, c0:c0 + w])
            sl = slice(c0, c0 + w)
            eng.tensor_tensor(t_sw[:, sl], t_sw[:, sl], t_sin[:, sl], OP.mult)
            eng.tensor_tensor(t_q[:, sl], t_q[:, sl], t_cos[:, sl], OP.mult)
            eng.tensor_tensor(t_q[:, sl], t_q[:, sl], t_sw[:, sl], OP.add)

        def wpass(wsl_pool, wdram, l, rhs, out_fn, n0=0, nn=NJ):
            """out[n] = sum_k W[l,k,n-block].T @ rhs_k for n in [n0, n0+nn);
            W streamed, psum-resident over n.  out_fn(n, ps) evicts."""
            pss = [psum() for _ in range(nn)]
            for k in range(NJ):
                wk = wsl_pool.tile([128, nn * 128], BF16, tag="wsl")
                nc.sync.dma_start(
                    wk[:], wdram.ap()[l, k * 128:(k + 1) * 128,
                                      n0 * 128:(n0 + nn) * 128])
                for n in range(nn):
                    nc.tensor.matmul(pss[n][:], wk[:, n * 128:(n + 1) * 128],
                                     rhs[:, k * TPC:(k + 1) * TPC],
                                     start=(k == 0), stop=(k == NJ - 1))
            for n in range(nn):
                out_fn(n, pss[n])

        def evict_bias(dst, dst_sl, bias_t, nb0=0, flip=0, vec_only=False):
            """psum + bias -> bf16 sbuf, alternating vector / scalar."""
            def f(nn, p):
                if vec_only or (nn + flip) % 2 == 0:
                    nc.vector.tensor_scalar(dst[:, dst_sl(nn)], p[:],
                                            bias_t[:, nb0 + nn:nb0 + nn + 1],
                                            None, OP.add)
                else:
                    nc.scalar.activation(dst[:, dst_sl(nn)], p[:], AF.Identity,
                                         bias=bias_t[:, nb0 + nn:nb0 + nn + 1])
            return f

        pend = [None]
        # ================= phase A: transformer layers =================
        with ExitStack() as actx:
            aconst = actx.enter_context(tc.tile_pool(name="aconst", bufs=1))
            kvp = actx.enter_context(tc.tile_pool(name="kvp", bufs=1))
            wk_ = actx.enter_context(tc.tile_pool(name="work", bufs=1))
            ap_ = actx.enter_context(tc.tile_pool(name="Ap", bufs=4))
            wsl = actx.enter_context(tc.tile_pool(name="wsl", bufs=6))
            h1p = actx.enter_context(tc.tile_pool(name="h1p", bufs=1))
            dvp = actx.enter_context(tc.tile_pool(name="dvp", bufs=4))

            t_cos = aconst.tile([128, NJ * TPC], BF16, tag="cos")
            nc.scalar.dma_start(t_cos[:], di["cosT"].ap())
            t_sin = aconst.tile([128, NJ * TPC], BF16, tag="sin")
            nc.gpsimd.dma_start(t_sin[:], di["sinS"].ap())
            t_dm = aconst.tile([128, 128], BF16, tag="dmask")
            nc.scalar.dma_start(t_dm[:], di["dmask4"].ap())

            # gathered K (feature-major, [j-tile, slot, key]) and
            # V ([slot, head-pair, feat]) -- keys rank-major within a slot
            t_Kg = kvp.tile([128, NJ * 8 * TB], BF16, tag="Kg")
            t_Vg = kvp.tile([128, 8 * NJ * TB], BF16, tag="Vg")

            for l in range(NLAYER):
                # --- per-layer bias/param tiles
                bt = {}
                for nm in ("bq_p", "bk_p", "bo_p", "b2_p", "g_p", "be_p",
                           "l2w_p", "l2b_p"):
                    bt[nm] = bias.tile([128, NJ], F32, tag=nm, name=f"bt_{nm}")
                    nc.sync.dma_start(bt[nm][:], di[nm].ap()[l])
                t_b1 = bias.tile([128, NJ1], F32, tag="b1")
                nc.sync.dma_start(t_b1[:], di["b1_p"].ap()[l])
                t_bvb = bias.tile([128, D], F32, tag="bvb")
                nc.sync.dma_start(t_bvb[:], di["bv_bc"].ap()[l])

                # --- LN1 (fused with the previous layer's W2 eviction+residual)
                t_xn = wk_.tile([128, NJ * TPC], BF16, tag="xn")
                layernorm(wk_, t_x, t_xn, bt["g_p"], bt["be_p"], pre=pend[0])
                pend[0] = None

                # --- K/V projection + AllGather in 3 head-group chunks
                t_k = wk_.tile([128, NJ * TPC], BF16, tag="k")
                kv_outs = []
                for c in range(3):
                    t_vc = dvp.tile([128, 2 * TPC], BF16, tag="vc",
                                    name=f"vc{l}_{c}")
                    # K projection for j-tiles 2c, 2c+1
                    wpass(wsl, di["Wk"], l, t_xn,
                          evict_bias(t_k,
                                     lambda n, c=c: slice((2 * c + n) * TPC,
                                                          (2 * c + n + 1) * TPC),
                                     bt["bk_p"], nb0=2 * c),
                          n0=2 * c, nn=2)
                    rope(wk_, t_k, t_cos, t_sin, 2 * c * TPC, 2 * TPC,
                         eng=nc.gpsimd)
                    kv_in = dram.tile([4 * 128 * TPC], BF16, tag=f"kv_in{c}",
                                      name=f"kvi{l}_{c}")
                    nc.scalar.dma_start(
                        kv_in[0:2 * 128 * TPC]
                        .rearrange("(j p t) -> p j t", j=2, p=128),
                        t_k[:, 2 * c * TPC:(2 * c + 2) * TPC]
                        .rearrange("p (j t) -> p j t", j=2))
                    # V projection for feature cols [256c, 256c+256)
                    psv = [psum(TPC) for _ in range(2)]
                    for k in range(NJ):
                        wvk = wsl.tile([128, TPC], BF16, tag="wsl",
                                       name=f"wv{l}_{c}_{k}")
                        nc.sync.dma_start(
                            wvk[:], di["Wv"].ap()[l, k * 128:(k + 1) * 128,
                                                  c * TPC:(c + 1) * TPC])
                        for tt in range(2):
                            lhs = t_xn[:, k * TPC + tt * TB:
                                       k * TPC + (tt + 1) * TB]
                            nc.tensor.matmul(psv[tt][:], lhs, wvk[:],
                                             start=(k == 0), stop=(k == NJ - 1))
                    for tt in range(2):
                        eng = nc.vector
                        eng.tensor_tensor(t_vc[:, tt * TPC:(tt + 1) * TPC],
                                          psv[tt][:],
                                          t_bvb[:, c * TPC:(c + 1) * TPC],
                                          OP.add)
                    nc.scalar.dma_start(
                        kv_in[2 * 128 * TPC:]
                        .rearrange("(tt p e) -> p tt e", tt=2, p=128),
                        t_vc[:].rearrange("p (tt e) -> p tt e", tt=2))
                    kv_out = dram.tile([4, 4 * 128 * TPC], BF16, tag=f"kv_out{c}",
                                       name=f"kvo{l}_{c}")
                    nc.gpsimd.collective_compute(
                        "AllGather", OP.bypass,
                        replica_groups=[[0, 1, 2, 3], [4, 5, 6, 7]],
                        ins=[kv_in[:].opt()], outs=[kv_out[:].opt()])
                    kv_outs.append(kv_out)

                # --- Q projection + RoPE (overlaps the collectives)
                t_q = wk_.tile([128, NJ * TPC], BF16, tag="q")
                wpass(wsl, di["Wq"], l, t_xn,
                      evict_bias(t_q, lambda n: slice(n * TPC, (n + 1) * TPC),
                                 bt["bq_p"], flip=1))
                for qc in range(3):
                    rope(wk_, t_q, t_cos, t_sin, 2 * qc * TPC, 2 * TPC)

                t_attT = wk_.tile([128, NJ * TPC], BF16, tag="attT")
                dps = [None, None, None]
                Rh = [None] * NJ

                def attention_hp(hp):
                    hA = [ap_.tile([128, AW], BF16, tag="A",
                                   name=f"A{l}_{hp}_{i}") for i in range(2)]
                    # QK into pair-group psums, exp-evict to A
                    for g in range(4):
                        for hi in range(2):
                            qg = psum(GW[g])
                            for s2 in range(2):
                                s = 2 * g + s2
                                off = 0 if s2 == 0 else WS[2 * g]
                                nc.tensor.matmul(
                                    qg[:, off:off + WS[s]],
                                    t_Kg[64 * hi:64 * hi + 64,
                                         (hp * 8 + s) * TB:(hp * 8 + s + 1) * TB],
                                    t_q[64 * hi:64 * hi + 64,
                                        hp * TPC + 32 * s:(hp + 1) * TPC],
                                    start=True, stop=True)
                            nc.scalar.activation(
                                hA[hi][:, 512 * g:512 * g + GW[g]], qg[:],
                                AF.Exp, scale=0.125)
                    # diagonal masks: even slots at 512g, odd at 256+448g
                    dv = t_dm[:].rearrange("p (g q) -> p g q", g=4)
                    for hi in range(2):
                        eng = nc.vector if (hp + hi) % 2 == 0 else nc.gpsimd
                        ev = hA[hi][:].rearrange(
                            "p (g q) -> p g q", g=4)[:, :, 0:32]
                        ov = hA[hi][:, 256:2048].rearrange(
                            "p (g q) -> p g q", g=4)[:, :, 0:32]
                        eng.tensor_tensor(ev, ev, dv, OP.mult)
                        eng.tensor_tensor(ov, ov, dv, OP.mult)
                    # AV (V-stationary, 2 heads col-packed) + denominators
                    d = hp // 2
                    if hp % 2 == 0:
                        dps[d] = psum(TPC)
                        nc.vector.memset(dps[d][:], 1.0)
                    c0 = 32 * (2 * (hp % 2))
                    avp = psum(TPC)
                    for s in range(8):
                        st, sp = (s == 0), (s == 7)
                        for hi in range(2):
                            asl = hA[hi][:, OS[s]:OS[s] + WS[s]]
                            nc.tensor.matmul(
                                avp[64 * hi:64 * hi + 64, 32 * s:TPC],
                                t_Vg[:, (s * NJ + hp) * TB + 64 * hi:
                                     (s * NJ + hp) * TB + 64 * hi + 64],
                                asl, start=st, stop=sp)
                        for hi in range(2):
                            asl = hA[hi][:, OS[s]:OS[s] + WS[s]]
                            nc.tensor.matmul(
                                dps[d][c0 + 32 * hi:c0 + 32 * hi + 1,
                                       32 * s:TPC],
                                t_onesbf[:], asl, start=st, stop=sp,
                                tile_position=(0, c0 + 32 * hi))
                    return avp

                def attT_evict(hp, avp):
                    d = hp // 2
                    divp = psum(TPC)
                    sel = t_sele if hp % 2 == 0 else t_selo
                    nc.tensor.matmul(divp[:], sel[:], Rh[d][:])
                    t_div = dvp.tile([128, TPC], BF16, tag="div",
                                     name=f"div{l}_{hp}")
                    if hp % 2 == 0:
                        nc.scalar.copy(t_div[:], divp[:])
                    else:
                        nc.vector.tensor_copy(t_div[:], divp[:])
                    nc.vector.tensor_tensor(t_attT[:, hp * TPC:(hp + 1) * TPC],
                                            avp[:], t_div[:], OP.mult)

                for c in range(3):
                    kv_out = kv_outs[c]
                    # load gathered K (rank-major keys within each slot)
                    kgv = t_Kg[:].rearrange("p (j s q) -> p j s q", j=NJ, s=8)
                    vgv = t_Vg[:].rearrange("p (s h q) -> p s h q", s=8, h=NJ)
                    for r in range(4):
                        eng = [nc.sync, nc.gpsimd, nc.sync, nc.gpsimd][r]
                        for j2 in range(2):
                            eng.dma_start(
                                kgv[:, 2 * c + j2, :, 32 * r:32 * r + 32],
                                kv_out[r, j2 * 128 * TPC:(j2 + 1) * 128 * TPC]
                                .rearrange("(p s t) -> p s t", p=128, s=8))
                        for tt in range(2):
                            eng2 = [nc.scalar, nc.sync, nc.gpsimd, nc.scalar][r]
                            vsrc = kv_out[r, (2 + tt) * 128 * TPC:
                                          (3 + tt) * 128 * TPC].rearrange(
                                "(b t h q) -> t b h q", b=4, t=32, h=2)
                            for h2 in range(2):
                                eng2.dma_start(
                                    vgv[32 * r:32 * r + 32, 4 * tt:4 * tt + 4,
                                        2 * c + h2, :],
                                    vsrc[:, :, h2, :])
                    av0 = attention_hp(2 * c)
                    av1 = attention_hp(2 * c + 1)
                    d = c
                    Rh[d] = dvp.tile([128, TPC], F32R, tag="Rh",
                                     name=f"Rh{l}_{d}")
                    with nc.allow_low_precision(reason="f32r recip for bcast"):
                        nc.vector.reciprocal(Rh[d][:], dps[d][:])
                    attT_evict(2 * c, av0)
                    attT_evict(2 * c + 1, av1)

                # --- Wo + residual fused into LN2 chunk prologue
                t_mo = wk_.tile([128, NJ * TPC], F32, tag="mmout")
                po_ = [psum() for _ in range(NJ)]
                for k in range(NJ):
                    wok = wsl.tile([128, NJ * 128], BF16, tag="wsl")
                    nc.sync.dma_start(wok[:],
                                      di["Wo"].ap()[l, k * 128:(k + 1) * 128, :])
                    for n in range(NJ):
                        nc.tensor.matmul(po_[n][:], wok[:, n * 128:(n + 1) * 128],
                                         t_attT[:, k * TPC:(k + 1) * TPC],
                                         start=(k == 0), stop=(k == NJ - 1))
                eb_o = evict_bias(t_mo, lambda n: slice(n * TPC, (n + 1) * TPC),
                                  bt["bo_p"], vec_only=True)

                def pre_o(j):
                    sl = slice(j * TPC, (j + 1) * TPC)
                    eb_o(j, po_[j])
                    nc.gpsimd.tensor_tensor(t_x[:, sl], t_x[:, sl], t_mo[:, sl],
                                            OP.add)

                # --- LN2 + MLP
                t_xn2 = wk_.tile([128, NJ * TPC], BF16, tag="xn")
                layernorm(wk_, t_x, t_xn2, bt["l2w_p"], bt["l2b_p"], pre=pre_o)

                t_h1 = h1p.tile([128, NJ1 * TPC], BF16, tag="h1")
                for g in range(4):
                    psg = [psum() for _ in range(NJ)]
                    for k in range(NJ):
                        w1k = wsl.tile([128, NJ * 128], BF16, tag="wsl")
                        nc.sync.dma_start(
                            w1k[:], di["W1"].ap()[l, k * 128:(k + 1) * 128,
                                                  g * D:(g + 1) * D])
                        for n in range(NJ):
                            nc.tensor.matmul(
                                psg[n][:], w1k[:, n * 128:(n + 1) * 128],
                                t_xn2[:, k * TPC:(k + 1) * TPC],
                                start=(k == 0), stop=(k == NJ - 1))
                    for n in range(NJ):
                        gn = g * NJ + n
                        nc.scalar.activation(
                            t_h1[:, gn * TPC:(gn + 1) * TPC], psg[n][:], AF.Gelu,
                            bias=t_b1[:, gn:gn + 1])

                p2_ = [psum() for _ in range(NJ)]
                for k in range(NJ1):
                    w2k = wsl.tile([128, NJ * 128], BF16, tag="wsl")
                    nc.sync.dma_start(w2k[:],
                                      di["W2"].ap()[l, k * 128:(k + 1) * 128, :])
                    for n in range(NJ):
                        nc.tensor.matmul(p2_[n][:], w2k[:, n * 128:(n + 1) * 128],
                                         t_h1[:, k * TPC:(k + 1) * TPC],
                                         start=(k == 0), stop=(k == NJ1 - 1))
                eb_2 = evict_bias(t_mo, lambda n: slice(n * TPC, (n + 1) * TPC),
                                  bt["b2_p"], flip=1, vec_only=True)

                def mk_pre2(psums, eb, mo):
                    def pre2(j):
                        sl = slice(j * TPC, (j + 1) * TPC)
                        eb(j, psums[j])
                        nc.gpsimd.tensor_tensor(t_x[:, sl], t_x[:, sl],
                                                mo[:, sl], OP.add)
                    return pre2
                pend[0] = mk_pre2(p2_, eb_2, t_mo)

        # ================= phase B: final LN + classifier =================
        with ExitStack() as bctx:
            bw = bctx.enter_context(tc.tile_pool(name="bw", bufs=1))
            hallp = bctx.enter_context(tc.tile_pool(name="hall", bufs=1))
            embp = bctx.enter_context(tc.tile_pool(name="embp", bufs=14))

            layernorm(bw, t_x, t_hT, t_lnw, t_lnb, pre=pend[0])
            pend[0] = None
            hag_in = dram.tile([D, TPC], BF16, tag="hag_in")
            nc.scalar.dma_start(
                hag_in[:].rearrange("(j p) t -> p j t", p=128),
                t_hT[:].rearrange("p (j t) -> p j t", j=NJ))
            hag_out = dram.tile([8 * D, TPC], BF16, tag="hag_out",
                                addr_space="Shared")
            nc.gpsimd.collective_compute(
                "AllGather", OP.bypass,
                replica_groups=[[0, 1, 2, 3, 4, 5, 6, 7]],
                ins=[hag_in[:].opt()], outs=[hag_out[:].opt()])

            t_hall = hallp.tile([128, 8 * NJ * TPC], BF16, tag="hall")
            hall4 = t_hall[:].rearrange("p (r j t) -> p r j t", r=8, j=NJ)
            for r in range(8):
                eng = nc.scalar if r % 2 == 0 else nc.gpsimd
                eng.dma_start(
                    hall4[:, r], hag_out[r * D:(r + 1) * D, :]
                    .rearrange("(j p) t -> p j t", p=128))

            for vc in range(NVC):
                ets = []
                for k in range(NJ):
                    et = embp.tile([128, VCHUNK], BF16, tag="emb",
                                   name=f"emb{vc}_{k}")
                    nc.sync.dma_start(
                        et[:], di["embT"].ap()[k * 128:(k + 1) * 128,
                                               vc * VCHUNK:(vc + 1) * VCHUNK])
                    ets.append(et)
                for mt in range(NMT):
                    r, hf = divmod(mt, 2)
                    pc = psum(VCHUNK)
                    for k in range(NJ):
                        nc.tensor.matmul(
                            pc[:],
                            t_hall[:, (r * NJ + k) * TPC + hf * TB:
                                   (r * NJ + k) * TPC + (hf + 1) * TB],
                            ets[k][:], start=(k == 0), stop=(k == NJ - 1))
                    so = embp.tile([128, VCHUNK], BF16, tag="clso",
                                   name=f"clso{vc}_{mt}")
                    if mt % 2 == 0:
                        nc.scalar.copy(so[:], pc[:])
                    else:
                        nc.vector.tensor_copy(so[:], pc[:])
                    oeng = nc.sync if mt % 2 == 0 else nc.gpsimd
                    oeng.dma_start(
                        out_logits.ap()[mt * 128:(mt + 1) * 128,
                                        vc * VCHUNK:(vc + 1) * VCHUNK], so[:])

    nc.compile()
    return nc


_NC = None


def _get_nc():
    global _NC
    if _NC is None:
        _NC = _build()
    return _NC


def _pack_fm(M):
    """[768, t] feature-major -> [128, 6*t] tile layout (row d=128*j+p)."""
    t = M.shape[1]
    return np.ascontiguousarray(
        M.reshape(NJ, 128, t).transpose(1, 0, 2).reshape(128, NJ * t),
        dtype=np.float32)


def _pack_pp(v):
    """per-feature vector [D'] -> per-partition [128, D'/128]."""
    return np.ascontiguousarray(v.reshape(-1, 128).T, dtype=np.float32)


def _core_pos(i):
    """core sub-index i -> global positions of its 256 local queries
    (chunk-major: local l = 32*b + t -> global 128*b + 32*i + t)."""
    ll = np.arange(TPC)
    return 128 * (ll // 32) + 32 * i + (ll % 32)


def _prep_in_maps(inputs):
    import ml_dtypes
    bf = ml_dtypes.bfloat16
    f32 = lambda a: np.ascontiguousarray(a, dtype=np.float32)
    f16 = lambda a: np.ascontiguousarray(a, dtype=bf)
    emb = f32(inputs["emb"])
    tok = np.asarray(inputs["input_token"]).astype(np.int64)
    x0 = emb[tok]                                    # [B, T, D]

    sel_e = np.zeros((128, 128), np.float32)
    sel_e[0, 0:64] = 1.0
    sel_e[32, 64:128] = 1.0
    sel_o = np.zeros((128, 128), np.float32)
    sel_o[64, 0:64] = 1.0
    sel_o[96, 64:128] = 1.0

    shared = {
        "Wq": f16(inputs["Wq"]), "Wk": f16(inputs["Wk"]),
        "Wv": f16(inputs["Wv"]), "Wo": f16(inputs["Wo"]),
        "W1": f16(inputs["W1"]), "W2": f16(inputs["W2"]),
        "onecol": np.full((128, 1), 1.0 / D, np.float32),
        "onesrow": np.ones((1, 128), np.float32),
        "onesbf": np.ones((128, 1), bf),
        "sel_e": sel_e, "sel_o": sel_o,
        "lnw_p": _pack_pp(f32(inputs["ln_w"])),
        "lnb_p": _pack_pp(f32(inputs["ln_b"])),
    }
    for nm, src in (("bq_p", "bq"), ("bk_p", "bk"), ("bo_p", "bo"),
                    ("b2_p", "b2"), ("g_p", "gamma"), ("be_p", "beta"),
                    ("l2w_p", "ln2_w"), ("l2b_p", "ln2_b")):
        shared[nm] = np.stack([_pack_pp(f32(inputs[src][l])) for l in range(L)])
    shared["b1_p"] = np.stack([_pack_pp(f32(inputs["b1"][l])) for l in range(L)])
    shared["bv_bc"] = np.stack(
        [np.tile(f32(inputs["bv"][l])[None, :], (128, 1)) for l in range(L)])

    inv = 1.0 / (10000.0 ** (np.arange(0, DK, 2, dtype=np.float32) / DK))
    embT_full = emb.T                                # [D, V]
    vpad = np.zeros((D, 8 * VC), np.float32)
    vpad[:, :V] = embT_full

    in_maps = []
    for c in range(8):
        beta, i = divmod(c, 4)
        pos = _core_pos(i)
        xc = x0[beta, pos]                           # [256, D]
        m = dict(shared)
        m["x0T"] = _pack_fm(xc.T)

        fr = pos[:, None].astype(np.float32) * inv[None, :]      # [256, 32]
        ang = np.concatenate([fr, fr], 1)                        # [256, 64]
        cosT = np.cos(ang).T                                     # [64, 256]
        sinT = np.sin(ang).T
        sinSg = sinT.copy()
        sinSg[:32] = -sinT[:32]
        m["cosT"] = np.ascontiguousarray(np.tile(cosT, (2, NJ))).astype(bf)
        m["sinS"] = np.ascontiguousarray(np.tile(sinSg, (2, NJ))).astype(bf)

        # diagonal mask: key row k (rank-major global order within block)
        # vs query col t of this core's 32-sub-block:  k <= 32*i + t
        km = np.arange(128)[:, None] <= (32 * i + np.arange(32))[None, :]
        m["dmask4"] = np.tile(km.astype(np.float32), (1, 4)).astype(bf)

        esl = np.zeros((D, VCP), np.float32)
        esl[:, :VC] = vpad[:, c * VC:(c + 1) * VC]
        m["embT"] = esl.astype(bf)
        in_maps.append(m)

    return in_maps


def _assemble(res):
    out = np.empty((B, T, 8 * VC), np.float32)
    for c in range(8):
        lr = np.asarray(res.results[c]["logits"]).astype(np.float32)
        lr = lr.reshape(NMT * 128, VCP)
        for r in range(8):
            beta, i = divmod(r, 4)
            pos = _core_pos(i)
            out[beta, pos, c * VC:(c + 1) * VC] = \
                lr[r * TPC:(r + 1) * TPC, :VC]
    return np.ascontiguousarray(out[:, :, :V])


def kernel(**inputs):
    nc = _get_nc()
    in_maps = _prep_in_maps(inputs)
    res = bass_utils.run_bass_kernel_spmd(nc, in_maps, core_ids=list(range(8)))
    return _assemble(res)


def run_traced(inputs, tmpdir):
    nc = _get_nc()
    in_maps = _prep_in_maps(inputs)
    return bass_utils.run_bass_kernel_spmd(
        nc, in_maps, core_ids=list(range(8)), trace=True, tmpdir=tmpdir)


# revision 17
# speedup vs baseline: 1.0077x; 1.0077x over previous
"""GPT forward (L=6, B=2, T=1024, D=768, H=12, V=50257) on 8 TRN2 NeuronCores.

Sharding: queries 32-interleaved (core i of each 4-core batch group owns the
i-th 32-token sub-block of every 128-token block), weights replicated.  This
makes block-causal skipping core-uniform: every core computes exactly the
9 causally-live (q-chunk, key-block) pair units instead of all 20.

Per layer the K/V AllGather is split into 3 head-group chunks that pipeline
with attention (chunk c feeds head-pairs 2c, 2c+1), keeping the PE busy
through the collective.  AV runs V-stationary and column-packed (two heads
per matmul, out = V.T @ A feature-major), so attention output needs no PE
transposes.  Softmax denominators come from ones-column matmuls packed into
spare PSUM partitions; LN rstd is computed as Exp(-0.5*Ln(var+eps)) so the
whole layer uses one ACT table set plus gelu.

All matmul operands are bf16; residual stream and LN stats stay fp32.
"""
import os
import numpy as np
from contextlib import ExitStack

import concourse.bass as bass
import concourse.tile as tile
import concourse.mybir as mybir
from concourse import bacc, bass_utils

F32 = mybir.dt.float32
F32R = mybir.dt.float32r
BF16 = mybir.dt.bfloat16
AF = mybir.ActivationFunctionType
OP = mybir.AluOpType

L, B, T, D, H, DK, V = 6, 2, 1024, 768, 12, 64, 50257
NB, TB, TPC = 8, 128, 256
NJ = D // 128                        # 6
NJ1 = 4 * D // 128                   # 24
VCHUNK = 512
NVC = 13
VCP = NVC * VCHUNK                   # 6656
VC = 6283                            # 8*6283 = 50264 >= V
EPS = 1e-5
NMT = 16
NLAYER = int(os.environ.get("KLAYERS", str(L)))

# slot layout inside an A tile: pair g at 512*g, tight within the pair
WS = [256 - 32 * s for s in range(8)]            # slot widths
OS = [0, 256, 512, 704, 1024, 1152, 1536, 1600]  # slot offsets
GW = [480, 352, 224, 96]                         # pair-group widths
AW = 2048                                        # A tile width (4 x 512)


def _build():
    nc = bacc.Bacc("TRN2", target_bir_lowering=False, debug=False)

    di = {}
    def din(name, shape, dt=F32R):
        di[name] = nc.dram_tensor(name, shape, dt, kind="ExternalInput")
        return di[name]

    din("x0T", [128, NJ * TPC])
    din("cosT", [128, NJ * TPC], BF16)
    din("sinS", [128, NJ * TPC], BF16)
    din("dmask4", [128, 4 * 32], BF16)
    din("onecol", [128, 1])
    din("onesrow", [1, 128])
    din("onesbf", [128, 1], BF16)
    din("sel_e", [128, 128])
    din("sel_o", [128, 128])
    din("embT", [D, VCP], BF16)
    for nm in ("Wq", "Wk", "Wv", "Wo"):
        din(nm, [L, D, D], BF16)
    din("W1", [L, D, 4 * D], BF16)
    din("W2", [L, 4 * D, D], BF16)
    for nm in ("bq_p", "bk_p", "bo_p", "b2_p", "g_p", "be_p", "l2w_p", "l2b_p"):
        din(nm, [L, 128, NJ], F32)
    din("b1_p", [L, 128, NJ1], F32)
    din("bv_bc", [L, 128, D], F32)
    din("lnw_p", [128, NJ], F32)
    din("lnb_p", [128, NJ], F32)

    out_logits = nc.dram_tensor("logits", [NMT * 128, VCP], BF16,
                                kind="ExternalOutput")

    with tile.TileContext(nc) as tc, ExitStack() as octx:
        const = octx.enter_context(tc.tile_pool(name="const", bufs=1))
        xpool = octx.enter_context(tc.tile_pool(name="x", bufs=1))
        small = octx.enter_context(tc.tile_pool(name="small", bufs=4))
        bias = octx.enter_context(tc.tile_pool(name="bias", bufs=2))
        pp = octx.enter_context(tc.tile_pool(name="pp", bufs=8, space="PSUM"))
        dram = octx.enter_context(tc.tile_pool(name="dram", bufs=2, space="DRAM"))

        t_ones = const.tile([128, 1], F32R, tag="ones")
        nc.sync.dma_start(t_ones[:], di["onecol"].ap())
        t_onesr = const.tile([1, 128], F32R, tag="onesr")
        nc.sync.dma_start(t_onesr[:], di["onesrow"].ap())
        t_onesbf = const.tile([128, 1], BF16, tag="onesbf")
        nc.sync.dma_start(t_onesbf[:], di["onesbf"].ap())
        t_sele = const.tile([128, 128], F32R, tag="sele")
        nc.sync.dma_start(t_sele[:], di["sel_e"].ap())
        t_selo = const.tile([128, 128], F32R, tag="selo")
        nc.sync.dma_start(t_selo[:], di["sel_o"].ap())
        t_lnw = const.tile([128, NJ], F32, tag="lnw")
        nc.sync.dma_start(t_lnw[:], di["lnw_p"].ap())
        t_lnb = const.tile([128, NJ], F32, tag="lnb")
        nc.sync.dma_start(t_lnb[:], di["lnb_p"].ap())
        t_eps = const.tile([1, 1], F32, tag="eps")
        nc.gpsimd.memset(t_eps[:], EPS)

        t_x = xpool.tile([128, NJ * TPC], F32R, tag="x")
        nc.sync.dma_start(t_x[:], di["x0T"].ap())
        t_hT = xpool.tile([128, NJ * TPC], BF16, tag="hT")

        pcnt = [0]

        def psum(w=TPC):
            pcnt[0] += 1
            return pp.tile([128, w], F32, tag="pp", name=f"ps{pcnt[0]}")

        def psum1(w=TPC):
            pcnt[0] += 1
            return pp.tile([1, w], F32, tag="pp", name=f"ps{pcnt[0]}")

        def layernorm(wpool, src, dst, gt, bt, pre=None):
            """feature-major LN: dst(bf16) = (src - mean)/std * g + b.
            pre(j) runs before chunk j's stats (fused residual eviction).
            rstd = exp(-0.5*ln(var+eps)) -- stays in the exp/ln ACT set."""
            t_sq = wpool.tile([128, NJ * TPC], F32R, tag="scratch6")
            p_s = psum1()
            p_q = psum1()
            for j in range(NJ):
                if pre is not None:
                    pre(j)
                sl = slice(j * TPC, (j + 1) * TPC)
                nc.gpsimd.tensor_tensor(t_sq[:, sl], src[:, sl], src[:, sl],
                                        OP.mult)
                nc.tensor.matmul(p_s[:], t_ones[:], src[:, sl],
                                 start=(j == 0), stop=(j == NJ - 1))
                nc.tensor.matmul(p_q[:], t_ones[:], t_sq[:, sl],
                                 start=(j == 0), stop=(j == NJ - 1))
            # ones vector holds 1/D, so p_s = mean and p_q = E[x^2] directly
            t_mean = small.tile([1, TPC], F32R, tag="mean")
            nc.vector.tensor_copy(t_mean[:], p_s[:])
            t_msq = small.tile([1, TPC], F32, tag="msq")
            nc.vector.tensor_tensor(t_msq[:], t_mean[:], p_s[:], OP.mult)
            t_var = small.tile([1, TPC], F32, tag="var")
            nc.vector.tensor_tensor(t_var[:], p_q[:], t_msq[:], OP.subtract)
            t_lv = small.tile([1, TPC], F32, tag="lv")
            nc.scalar.activation(t_lv[:], t_var[:], AF.Ln, bias=t_eps[:])
            t_rstd = small.tile([1, TPC], F32R, tag="rstd")
            with nc.allow_low_precision(reason="f32r view of f32 for PE bcast"):
                nc.scalar.activation(t_rstd[:], t_lv[:], AF.Exp, scale=-0.5)
            t_mb = small.tile([128, TPC], F32, tag="mb")
            p_mb = psum()
            nc.tensor.matmul(p_mb[:], t_onesr[:], t_mean[:])
            nc.vector.tensor_copy(t_mb[:], p_mb[:])
            t_rb = small.tile([128, TPC], F32, tag="rb")
            p_rb = psum()
            nc.tensor.matmul(p_rb[:], t_onesr[:], t_rstd[:])
            nc.vector.tensor_copy(t_rb[:], p_rb[:])
            t_c = wpool.tile([128, NJ * TPC], F32, tag="lnc")
            for j in range(NJ):
                sl = slice(j * TPC, (j + 1) * TPC)
                eng = nc.vector if j % 2 == 0 else nc.gpsimd
                eng.tensor_tensor(t_c[:, sl], src[:, sl], t_mb[:], OP.subtract)
                eng.tensor_tensor(dst[:, sl], t_c[:, sl], t_rb[:], OP.mult)
                eng.tensor_scalar(dst[:, sl], dst[:, sl], gt[:, j:j + 1],
                                  bt[:, j:j + 1], OP.mult, OP.add)

        def rope(wpool, t_q, t_cos, t_sin, c0, w, eng=None):
            """in-place RoPE on feature-major bf16 cols [c0, c0+w).
            Partition swap via SBUF->SBUF DMAs on two queues."""
            eng = eng or nc.vector
            t_sw = wpool.tile([128, NJ * TPC], BF16, tag="ropesw")
            nc.sync.dma_start(t_sw[0:32, c0:c0 + w], t_q[32:64, c0:c0 + w])
            nc.gpsimd.dma_start(t_sw[32:64, c0:c0 + w], t_q[0:32, c0:c0 + w])
            nc.gpsimd.dma_start(t_sw[64:96, c0:c0 + w], t_q[96:128, c0:c0 + w])
            nc.sync.dma_start(t_sw[96:128, c0:c0 + w], t_q[64:96, c0:c0 + w])
            sl = slice(c0, c0 + w)
            eng.tensor_tensor(t_sw[:, sl], t_sw[:, sl], t_sin[:, sl], OP.mult)
            eng.tensor_tensor(t_q[:, sl], t_q[:, sl], t_cos[:, sl], OP.mult)
            eng.tensor_tensor(t_q[:, sl], t_q[:, sl], t_sw[:, sl], OP.add)

        def wpass(wsl_pool, wdram, l, rhs, out_fn, n0=0, nn=NJ):
            """out[n] = sum_k W[l,k,n-block].T @ rhs_k for n in [n0, n0+nn);
            W streamed, psum-resident over n.  out_fn(n, ps) evicts."""
            pss = [psum() for _ in range(nn)]
            for k in range(NJ):
                wk = wsl_pool.tile([128, nn * 128], BF16, tag="wsl")
                nc.sync.dma_start(
                    wk[:], wdram.ap()[l, k * 128:(k + 1) * 128,
                                      n0 * 128:(n0 + nn) * 128])
                for n in range(nn):
                    nc.tensor.matmul(pss[n][:], wk[:, n * 128:(n + 1) * 128],
                                     rhs[:, k * TPC:(k + 1) * TPC],
                                     start=(k == 0), stop=(k == NJ - 1))
            for n in range(nn):
                out_fn(n, pss[n])

        def evict_bias(dst, dst_sl, bias_t, nb0=0, flip=0, vec_only=False):
            """psum + bias -> bf16 sbuf, alternating vector / scalar."""
            def f(nn, p):
                if vec_only or (nn + flip) % 2 == 0:
                    nc.vector.tensor_scalar(dst[:, dst_sl(nn)], p[:],
                                            bias_t[:, nb0 + nn:nb0 + nn + 1],
                                            None, OP.add)
                else:
                    nc.scalar.activation(dst[:, dst_sl(nn)], p[:], AF.Identity,
                                         bias=bias_t[:, nb0 + nn:nb0 + nn + 1])
            return f

        pend = [None]
        # ================= phase A: transformer layers =================
        with ExitStack() as actx:
            aconst = actx.enter_context(tc.tile_pool(name="aconst", bufs=1))
            kvp = actx.enter_context(tc.tile_pool(name="kvp", bufs=1))
            wk_ = actx.enter_context(tc.tile_pool(name="work", bufs=1))
            ap_ = actx.enter_context(tc.tile_pool(name="Ap", bufs=4))
            wsl = actx.enter_context(tc.tile_pool(name="wsl", bufs=6))
            h1p = actx.enter_context(tc.tile_pool(name="h1p", bufs=1))
            dvp = actx.enter_context(tc.tile_pool(name="dvp", bufs=4))

            t_cos = aconst.tile([128, NJ * TPC], BF16, tag="cos")
            nc.scalar.dma_start(t_cos[:], di["cosT"].ap())
            t_sin = aconst.tile([128, NJ * TPC], BF16, tag="sin")
            nc.gpsimd.dma_start(t_sin[:], di["sinS"].ap())
            t_dm = aconst.tile([128, 128], BF16, tag="dmask")
            nc.scalar.dma_start(t_dm[:], di["dmask4"].ap())

            # gathered K (feature-major, [j-tile, slot, key]) and
            # V ([slot, head-pair, feat]) -- keys rank-major within a slot
            t_Kg = kvp.tile([128, NJ * 8 * TB], BF16, tag="Kg")
            t_Vg = kvp.tile([128, 8 * NJ * TB], BF16, tag="Vg")


            for l in range(NLAYER):
                # --- per-layer bias/param tiles
                bt = {}
                for nm in ("bq_p", "bk_p", "bo_p", "b2_p", "g_p", "be_p",
                           "l2w_p", "l2b_p"):
                    bt[nm] = bias.tile([128, NJ], F32, tag=nm, name=f"bt_{nm}")
                    nc.sync.dma_start(bt[nm][:], di[nm].ap()[l])
                t_b1 = bias.tile([128, NJ1], F32, tag="b1")
                nc.sync.dma_start(t_b1[:], di["b1_p"].ap()[l])
                t_bvb = bias.tile([128, D], F32, tag="bvb")
                nc.sync.dma_start(t_bvb[:], di["bv_bc"].ap()[l])

                # --- LN1 (fused with the previous layer's W2 eviction+residual)
                t_xn = wk_.tile([128, NJ * TPC], BF16, tag="xn")
                layernorm(wk_, t_x, t_xn, bt["g_p"], bt["be_p"], pre=pend[0])
                pend[0] = None

                # --- K/V projection + AllGather in 3 head-group chunks
                t_k = wk_.tile([128, NJ * TPC], BF16, tag="k")
                kv_outs = []
                for c in range(3):
                    t_vc = dvp.tile([128, 2 * TPC], BF16, tag="vc",
                                    name=f"vc{l}_{c}")
                    # K projection for j-tiles 2c, 2c+1
                    wpass(wsl, di["Wk"], l, t_xn,
                          evict_bias(t_k,
                                     lambda n, c=c: slice((2 * c + n) * TPC,
                                                          (2 * c + n + 1) * TPC),
                                     bt["bk_p"], nb0=2 * c),
                          n0=2 * c, nn=2)
                    rope(wk_, t_k, t_cos, t_sin, 2 * c * TPC, 2 * TPC,
                         eng=nc.gpsimd)
                    kv_in = dram.tile([4 * 128 * TPC], BF16, tag=f"kv_in{c}",
                                      name=f"kvi{l}_{c}")
                    nc.scalar.dma_start(
                        kv_in[0:2 * 128 * TPC]
                        .rearrange("(j p t) -> p j t", j=2, p=128),
                        t_k[:, 2 * c * TPC:(2 * c + 2) * TPC]
                        .rearrange("p (j t) -> p j t", j=2))
                    # V projection for feature cols [256c, 256c+256)
                    psv = [psum(TPC) for _ in range(2)]
                    for k in range(NJ):
                        wvk = wsl.tile([128, TPC], BF16, tag="wsl",
                                       name=f"wv{l}_{c}_{k}")
                        nc.sync.dma_start(
                            wvk[:], di["Wv"].ap()[l, k * 128:(k + 1) * 128,
                                                  c * TPC:(c + 1) * TPC])
                        for tt in range(2):
                            lhs = t_xn[:, k * TPC + tt * TB:
                                       k * TPC + (tt + 1) * TB]
                            nc.tensor.matmul(psv[tt][:], lhs, wvk[:],
                                             start=(k == 0), stop=(k == NJ - 1))
                    for tt in range(2):
                        eng = nc.vector
                        eng.tensor_tensor(t_vc[:, tt * TPC:(tt + 1) * TPC],
                                          psv[tt][:],
                                          t_bvb[:, c * TPC:(c + 1) * TPC],
                                          OP.add)
                    nc.scalar.dma_start(
                        kv_in[2 * 128 * TPC:]
                        .rearrange("(tt p e) -> p tt e", tt=2, p=128),
                        t_vc[:].rearrange("p (tt e) -> p tt e", tt=2))
                    kv_out = dram.tile([4, 4 * 128 * TPC], BF16, tag=f"kv_out{c}",
                                       name=f"kvo{l}_{c}")
                    nc.gpsimd.collective_compute(
                        "AllGather", OP.bypass,
                        replica_groups=[[0, 1, 2, 3], [4, 5, 6, 7]],
                        ins=[kv_in[:].opt()], outs=[kv_out[:].opt()])
                    kv_outs.append(kv_out)

                # --- Q projection + RoPE (overlaps the collectives)
                t_q = wk_.tile([128, NJ * TPC], BF16, tag="q")
                wpass(wsl, di["Wq"], l, t_xn,
                      evict_bias(t_q, lambda n: slice(n * TPC, (n + 1) * TPC),
                                 bt["bq_p"], flip=1))
                for qc in range(3):
                    rope(wk_, t_q, t_cos, t_sin, 2 * qc * TPC, 2 * TPC)

                t_attT = wk_.tile([128, NJ * TPC], BF16, tag="attT")
                dps = [None, None, None]
                Rh = [None] * NJ

                def attention_hp(hp):
                    hA = [ap_.tile([128, AW], BF16, tag="A",
                                   name=f"A{l}_{hp}_{i}") for i in range(2)]
                    # QK into pair-group psums, exp-evict to A
                    for g in range(4):
                        for hi in range(2):
                            qg = psum(GW[g])
                            for s2 in range(2):
                                s = 2 * g + s2
                                off = 0 if s2 == 0 else WS[2 * g]
                                nc.tensor.matmul(
                                    qg[:, off:off + WS[s]],
                                    t_Kg[64 * hi:64 * hi + 64,
                                         (hp * 8 + s) * TB:(hp * 8 + s + 1) * TB],
                                    t_q[64 * hi:64 * hi + 64,
                                        hp * TPC + 32 * s:(hp + 1) * TPC],
                                    start=True, stop=True)
                            nc.scalar.activation(
                                hA[hi][:, 512 * g:512 * g + GW[g]], qg[:],
                                AF.Exp, scale=0.125)
                    # diagonal masks: even slots at 512g, odd at 256+448g
                    dv = t_dm[:].rearrange("p (g q) -> p g q", g=4)
                    for hi in range(2):
                        eng = nc.vector if (hp + hi) % 2 == 0 else nc.gpsimd
                        ev = hA[hi][:].rearrange(
                            "p (g q) -> p g q", g=4)[:, :, 0:32]
                        ov = hA[hi][:, 256:2048].rearrange(
                            "p (g q) -> p g q", g=4)[:, :, 0:32]
                        eng.tensor_tensor(ev, ev, dv, OP.mult)
                        eng.tensor_tensor(ov, ov, dv, OP.mult)
                    # AV (V-stationary, 2 heads col-packed) + denominators
                    d = hp // 2
                    if hp % 2 == 0:
                        dps[d] = psum(TPC)
                        nc.vector.memset(dps[d][:], 1.0)
                    c0 = 32 * (2 * (hp % 2))
                    avp = psum(TPC)
                    for s in range(8):
                        st, sp = (s == 0), (s == 7)
                        for hi in range(2):
                            asl = hA[hi][:, OS[s]:OS[s] + WS[s]]
                            nc.tensor.matmul(
                                avp[64 * hi:64 * hi + 64, 32 * s:TPC],
                                t_Vg[:, (s * NJ + hp) * TB + 64 * hi:
                                     (s * NJ + hp) * TB + 64 * hi + 64],
                                asl, start=st, stop=sp)
                        for hi in range(2):
                            asl = hA[hi][:, OS[s]:OS[s] + WS[s]]
                            nc.tensor.matmul(
                                dps[d][c0 + 32 * hi:c0 + 32 * hi + 1,
                                       32 * s:TPC],
                                t_onesbf[:], asl, start=st, stop=sp,
                                tile_position=(0, c0 + 32 * hi))
                    return avp

                def attT_evict(hp, avp):
                    d = hp // 2
                    divp = psum(TPC)
                    sel = t_sele if hp % 2 == 0 else t_selo
                    nc.tensor.matmul(divp[:], sel[:], Rh[d][:])
                    t_div = dvp.tile([128, TPC], BF16, tag="div",
                                     name=f"div{l}_{hp}")
                    if hp % 2 == 0:
                        nc.scalar.copy(t_div[:], divp[:])
                    else:
                        nc.vector.tensor_copy(t_div[:], divp[:])
                    nc.vector.tensor_tensor(t_attT[:, hp * TPC:(hp + 1) * TPC],
                                            avp[:], t_div[:], OP.mult)

                t_mo = wk_.tile([128, NJ * TPC], F32, tag="mmout")
                for c in range(3):
                    kv_out = kv_outs[c]
                    # load gathered K (rank-major keys within each slot)
                    # and V (rank-major key partitions)
                    kgv = t_Kg[:].rearrange("p (j s q) -> p j s q", j=NJ, s=8)
                    vgv = t_Vg[:].rearrange("p (s hq) -> p s hq", s=8)
                    for r in range(4):
                        eng = [nc.sync, nc.gpsimd, nc.sync, nc.gpsimd][r]
                        for j2 in range(2):
                            eng.dma_start(
                                kgv[:, 2 * c + j2, :, 32 * r:32 * r + 32],
                                kv_out[r, j2 * 128 * TPC:(j2 + 1) * 128 * TPC]
                                .rearrange("(p s t) -> p s t", p=128, s=8))
                        for tt in range(2):
                            eng2 = [nc.scalar, nc.sync, nc.gpsimd, nc.scalar][r]
                            eng2.dma_start(
                                vgv[32 * r:32 * r + 32, 4 * tt:4 * tt + 4,
                                    2 * c * TB:(2 * c + 2) * TB],
                                kv_out[r, (2 + tt) * 128 * TPC:
                                       (3 + tt) * 128 * TPC]
                                .rearrange("(b t e) -> t b e", b=4, t=32))
                    av0 = attention_hp(2 * c)
                    av1 = attention_hp(2 * c + 1)
                    d = c
                    Rh[d] = dvp.tile([128, TPC], F32R, tag="Rh",
                                     name=f"Rh{l}_{d}")
                    with nc.allow_low_precision(reason="f32r recip for bcast"):
                        nc.vector.reciprocal(Rh[d][:], dps[d][:])
                    attT_evict(2 * c, av0)
                    attT_evict(2 * c + 1, av1)
                    # Wo contribution of this chunk's two attT j-tiles,
                    # accumulated into t_mo in SBUF (bias folded on chunk 0)
                    pwo = [psum() for _ in range(NJ)]
                    for ki in range(2):
                        k = 2 * c + ki
                        wok = wsl.tile([128, NJ * 128], BF16, tag="wsl")
                        nc.sync.dma_start(
                            wok[:], di["Wo"].ap()[l, k * 128:(k + 1) * 128, :])
                        for n in range(NJ):
                            nc.tensor.matmul(
                                pwo[n][:], wok[:, n * 128:(n + 1) * 128],
                                t_attT[:, k * TPC:(k + 1) * TPC],
                                start=(ki == 0), stop=(ki == 1))
                    for n in range(NJ):
                        msl = t_mo[:, n * TPC:(n + 1) * TPC]
                        if c == 0:
                            if n % 2 == 0:
                                nc.vector.tensor_scalar(
                                    msl, pwo[n][:], bt["bo_p"][:, n:n + 1],
                                    None, OP.add)
                            else:
                                nc.scalar.add(msl, pwo[n][:],
                                              bt["bo_p"][:, n:n + 1])
                        else:
                            nc.vector.tensor_tensor(msl, msl, pwo[n][:],
                                                    OP.add)

                def pre_o(j):
                    sl = slice(j * TPC, (j + 1) * TPC)
                    nc.gpsimd.tensor_tensor(t_x[:, sl], t_x[:, sl], t_mo[:, sl],
                                            OP.add)

                # --- LN2 + MLP
                t_xn2 = wk_.tile([128, NJ * TPC], BF16, tag="xn")
                layernorm(wk_, t_x, t_xn2, bt["l2w_p"], bt["l2b_p"], pre=pre_o)

                t_h1 = h1p.tile([128, NJ1 * TPC], BF16, tag="h1")
                for g in range(4):
                    psg = [psum() for _ in range(NJ)]
                    for k in range(NJ):
                        w1k = wsl.tile([128, NJ * 128], BF16, tag="wsl")
                        nc.sync.dma_start(
                            w1k[:], di["W1"].ap()[l, k * 128:(k + 1) * 128,
                                                  g * D:(g + 1) * D])
                        for n in range(NJ):
                            nc.tensor.matmul(
                                psg[n][:], w1k[:, n * 128:(n + 1) * 128],
                                t_xn2[:, k * TPC:(k + 1) * TPC],
                                start=(k == 0), stop=(k == NJ - 1))
                    for n in range(NJ):
                        gn = g * NJ + n
                        nc.scalar.activation(
                            t_h1[:, gn * TPC:(gn + 1) * TPC], psg[n][:], AF.Gelu,
                            bias=t_b1[:, gn:gn + 1])

                p2_ = [psum() for _ in range(NJ)]
                for k in range(NJ1):
                    w2k = wsl.tile([128, NJ * 128], BF16, tag="wsl")
                    nc.sync.dma_start(w2k[:],
                                      di["W2"].ap()[l, k * 128:(k + 1) * 128, :])
                    for n in range(NJ):
                        nc.tensor.matmul(p2_[n][:], w2k[:, n * 128:(n + 1) * 128],
                                         t_h1[:, k * TPC:(k + 1) * TPC],
                                         start=(k == 0), stop=(k == NJ1 - 1))
                eb_2 = evict_bias(t_mo, lambda n: slice(n * TPC, (n + 1) * TPC),
                                  bt["b2_p"], flip=1, vec_only=True)

                def mk_pre2(psums, eb, mo):
                    def pre2(j):
                        sl = slice(j * TPC, (j + 1) * TPC)
                        eb(j, psums[j])
                        nc.gpsimd.tensor_tensor(t_x[:, sl], t_x[:, sl],
                                                mo[:, sl], OP.add)
                    return pre2
                pend[0] = mk_pre2(p2_, eb_2, t_mo)

        # ================= phase B: final LN + classifier =================
        with ExitStack() as bctx:
            bw = bctx.enter_context(tc.tile_pool(name="bw", bufs=1))
            hallp = bctx.enter_context(tc.tile_pool(name="hall", bufs=1))
            embp = bctx.enter_context(tc.tile_pool(name="embp", bufs=14))

            layernorm(bw, t_x, t_hT, t_lnw, t_lnb, pre=pend[0])
            pend[0] = None
            hag_in = dram.tile([D, TPC], BF16, tag="hag_in")
            nc.scalar.dma_start(
                hag_in[:].rearrange("(j p) t -> p j t", p=128),
                t_hT[:].rearrange("p (j t) -> p j t", j=NJ))
            hag_out = dram.tile([8 * D, TPC], BF16, tag="hag_out",
                                addr_space="Shared")
            nc.gpsimd.collective_compute(
                "AllGather", OP.bypass,
                replica_groups=[[0, 1, 2, 3, 4, 5, 6, 7]],
                ins=[hag_in[:].opt()], outs=[hag_out[:].opt()])

            t_hall = hallp.tile([128, 8 * NJ * TPC], BF16, tag="hall")
            hall4 = t_hall[:].rearrange("p (r j t) -> p r j t", r=8, j=NJ)
            for r in range(8):
                eng = nc.scalar if r % 2 == 0 else nc.gpsimd
                eng.dma_start(
                    hall4[:, r], hag_out[r * D:(r + 1) * D, :]
                    .rearrange("(j p) t -> p j t", p=128))

            for vc in range(NVC):
                ets = []
                for k in range(NJ):
                    et = embp.tile([128, VCHUNK], BF16, tag="emb",
                                   name=f"emb{vc}_{k}")
                    nc.sync.dma_start(
                        et[:], di["embT"].ap()[k * 128:(k + 1) * 128,
                                               vc * VCHUNK:(vc + 1) * VCHUNK])
                    ets.append(et)
                for mt in range(NMT):
                    r, hf = divmod(mt, 2)
                    pc = psum(VCHUNK)
                    for k in range(NJ):
                        nc.tensor.matmul(
                            pc[:],
                            t_hall[:, (r * NJ + k) * TPC + hf * TB:
                                   (r * NJ + k) * TPC + (hf + 1) * TB],
                            ets[k][:], start=(k == 0), stop=(k == NJ - 1))
                    so = embp.tile([128, VCHUNK], BF16, tag="clso",
                                   name=f"clso{vc}_{mt}")
                    if mt % 2 == 0:
                        nc.scalar.copy(so[:], pc[:])
                    else:
                        nc.vector.tensor_copy(so[:], pc[:])
                    oeng = nc.sync if mt % 2 == 0 else nc.gpsimd
                    oeng.dma_start(
                        out_logits.ap()[mt * 128:(mt + 1) * 128,
                                        vc * VCHUNK:(vc + 1) * VCHUNK], so[:])

    nc.compile()
    return nc


_NC = None


def _get_nc():
    global _NC
    if _NC is None:
        _NC = _build()
    return _NC


def _pack_fm(M):
    """[768, t] feature-major -> [128, 6*t] tile layout (row d=128*j+p)."""
    t = M.shape[1]
    return np.ascontiguousarray(
        M.reshape(NJ, 128, t).transpose(1, 0, 2).reshape(128, NJ * t),
        dtype=np.float32)


def _pack_pp(v):
    """per-feature vector [D'] -> per-partition [128, D'/128]."""
    return np.ascontiguousarray(v.reshape(-1, 128).T, dtype=np.float32)


def _core_pos(i):
    """core sub-index i -> global positions of its 256 local queries
    (chunk-major: local l = 32*b + t -> global 128*b + 32*i + t)."""
    ll = np.arange(TPC)
    return 128 * (ll // 32) + 32 * i + (ll % 32)


def _prep_in_maps(inputs):
    import ml_dtypes
    bf = ml_dtypes.bfloat16
    f32 = lambda a: np.ascontiguousarray(a, dtype=np.float32)
    f16 = lambda a: np.ascontiguousarray(a, dtype=bf)
    emb = f32(inputs["emb"])
    tok = np.asarray(inputs["input_token"]).astype(np.int64)
    x0 = emb[tok]                                    # [B, T, D]

    sel_e = np.zeros((128, 128), np.float32)
    sel_e[0, 0:64] = 1.0
    sel_e[32, 64:128] = 1.0
    sel_o = np.zeros((128, 128), np.float32)
    sel_o[64, 0:64] = 1.0
    sel_o[96, 64:128] = 1.0

    shared = {
        "Wq": f16(inputs["Wq"]), "Wk": f16(inputs["Wk"]),
        "Wv": f16(inputs["Wv"]), "Wo": f16(inputs["Wo"]),
        "W1": f16(inputs["W1"]), "W2": f16(inputs["W2"]),
        "onecol": np.full((128, 1), 1.0 / D, np.float32),
        "onesrow": np.ones((1, 128), np.float32),
        "onesbf": np.ones((128, 1), bf),
        "sel_e": sel_e, "sel_o": sel_o,
        "lnw_p": _pack_pp(f32(inputs["ln_w"])),
        "lnb_p": _pack_pp(f32(inputs["ln_b"])),
    }
    for nm, src in (("bq_p", "bq"), ("bk_p", "bk"), ("bo_p", "bo"),
                    ("b2_p", "b2"), ("g_p", "gamma"), ("be_p", "beta"),
                    ("l2w_p", "ln2_w"), ("l2b_p", "ln2_b")):
        shared[nm] = np.stack([_pack_pp(f32(inputs[src][l])) for l in range(L)])
    shared["b1_p"] = np.stack([_pack_pp(f32(inputs["b1"][l])) for l in range(L)])
    shared["bv_bc"] = np.stack(
        [np.tile(f32(inputs["bv"][l])[None, :], (128, 1)) for l in range(L)])

    inv = 1.0 / (10000.0 ** (np.arange(0, DK, 2, dtype=np.float32) / DK))
    embT_full = emb.T                                # [D, V]
    vpad = np.zeros((D, 8 * VC), np.float32)
    vpad[:, :V] = embT_full

    in_maps = []
    for c in range(8):
        beta, i = divmod(c, 4)
        pos = _core_pos(i)
        xc = x0[beta, pos]                           # [256, D]
        m = dict(shared)
        m["x0T"] = _pack_fm(xc.T)

        fr = pos[:, None].astype(np.float32) * inv[None, :]      # [256, 32]
        ang = np.concatenate([fr, fr], 1)                        # [256, 64]
        cosT = np.cos(ang).T                                     # [64, 256]
        sinT = np.sin(ang).T
        sinSg = sinT.copy()
        sinSg[:32] = -sinT[:32]
        m["cosT"] = np.ascontiguousarray(np.tile(cosT, (2, NJ))).astype(bf)
        m["sinS"] = np.ascontiguousarray(np.tile(sinSg, (2, NJ))).astype(bf)

        # diagonal mask: key row k (rank-major global order within block)
        # vs query col t of this core's 32-sub-block:  k <= 32*i + t
        km = np.arange(128)[:, None] <= (32 * i + np.arange(32))[None, :]
        m["dmask4"] = np.tile(km.astype(np.float32), (1, 4)).astype(bf)

        esl = np.zeros((D, VCP), np.float32)
        esl[:, :VC] = vpad[:, c * VC:(c + 1) * VC]
        m["embT"] = esl.astype(bf)
        in_maps.append(m)

    return in_maps


def _assemble(res):
    out = np.empty((B, T, 8 * VC), np.float32)
    for c in range(8):
        lr = np.asarray(res.results[c]["logits"]).astype(np.float32)
        lr = lr.reshape(NMT * 128, VCP)
        for r in range(8):
            beta, i = divmod(r, 4)
            pos = _core_pos(i)
            out[beta, pos, c * VC:(c + 1) * VC] = \
                lr[r * TPC:(r + 1) * TPC, :VC]
    return np.ascontiguousarray(out[:, :, :V])


def kernel(**inputs):
    nc = _get_nc()
    in_maps = _prep_in_maps(inputs)
    res = bass_utils.run_bass_kernel_spmd(nc, in_maps, core_ids=list(range(8)))
    return _assemble(res)


def run_traced(inputs, tmpdir):
    nc = _get_nc()
    in_maps = _prep_in_maps(inputs)
    return bass_utils.run_bass_kernel_spmd(
        nc, in_maps, core_ids=list(range(8)), trace=True, tmpdir=tmpdir)
